# revision 10
# baseline (speedup 1.0000x reference)
"""MoE attention kernel for Trainium2 (8 NeuronCores via bass/Tile).

Sharding: core c -> (expert e = c % 4, batch b = c // 4). Each core computes
its expert's full attention for its batch, applies the sigmoid gate, and the
gated partial outputs are AllReduced within each batch group {0..3}, {4..7}.

All matmuls run in bf16 (fp32 PSUM accumulation). Layouts:
  - x is fed transposed per batch: xT [D, S]
  - weights are fed transposed: w*T [D_in, D_out]
  - q/k are computed in natural [s, d] layout (for layernorm + rope), then
    transposed on the PE to qT/kT [d, s] for the attention matmuls
  - attention computes P^T = exp(scoresT) [sk, sq]; an extra all-ones column
    in the stationary [v | 1] operand accumulates the softmax denominator
  - output projection produces outT [D, S]; host transposes back
"""
import sys
import numpy as np

sys.path.insert(0, "/opt/trn_rl_repo")

import ml_dtypes  # noqa: E402

BF16_NP = ml_dtypes.bfloat16

# problem config (full size, hardcoded for the grader)
B, S, D, E, H = 2, 2048, 1024, 4, 16
HD = 64
N_CORES = 8
EPS = 1e-5


def _host_prep(inputs, cfg):
    """Build per-core input maps (numpy only)."""
    B, S, D, E, H = cfg["B"], cfg["S"], cfg["D"], cfg["E"], cfg["H"]
    x = np.asarray(inputs["x"], np.float32)
    fc = np.asarray(inputs["freqs_cos"], np.float32)  # [S, HD//2]
    fs = np.asarray(inputs["freqs_sin"], np.float32)
    wq, wk, wv, wo = (np.asarray(inputs[n], np.float32) for n in ("wq", "wk", "wv", "wo"))
    qg, qb = np.asarray(inputs["q_gamma"], np.float32), np.asarray(inputs["q_beta"], np.float32)
    kg, kb = np.asarray(inputs["k_gamma"], np.float32), np.asarray(inputs["k_beta"], np.float32)
    gw, gb = np.asarray(inputs["gate_w"], np.float32), np.asarray(inputs["gate_b"], np.float32)

    # expanded rope tables [S, D]: cos/sin duplicated into feature pairs, tiled over heads
    nh2 = D // (2 * fc.shape[1])  # number of head-blocks the [S, hd] pattern tiles over
    cos2 = np.repeat(fc, 2, axis=1)  # [S, hd]
    sin2 = np.repeat(fs, 2, axis=1)
    sgn = np.tile(np.array([-1.0, 1.0], np.float32), fc.shape[1])  # [-s,+s] pairs
    cos_full = np.tile(cos2, (1, nh2))  # [S, D]
    ssin_full = np.tile(sin2 * sgn[None, :], (1, nh2))  # signed sin [S, D]

    def swap_pairs(v):
        return v.reshape(-1, 2)[:, ::-1].reshape(-1)

    in_maps = []
    for c in range(N_CORES):
        e, b = c % E, c // E
        # fold gamma into rope tables: C' = cos * gamma ; S' = ssin * gamma[swap]
        cq = (cos_full * qg[e][None, :]).astype(BF16_NP)
        sq = (ssin_full * swap_pairs(qg[e])[None, :]).astype(BF16_NP)
        ck = (cos_full * kg[e][None, :]).astype(BF16_NP)
        sk_ = (ssin_full * swap_pairs(kg[e])[None, :]).astype(BF16_NP)
        m = {
            "xT": np.ascontiguousarray(x[b].T).astype(BF16_NP),
            "wqT": np.ascontiguousarray(wq[e].T).astype(BF16_NP),
            "wkT": np.ascontiguousarray(wk[e].T).astype(BF16_NP),
            "wvT": np.ascontiguousarray(wv[e].T).astype(BF16_NP),
            "woT": np.ascontiguousarray(wo[e].T).astype(BF16_NP),
            "gw": np.ascontiguousarray(gw[e][:, None]).astype(BF16_NP),
            "negb": np.full((1, 1), -gb[e], np.float32),
            "cq": cq, "sq": sq, "ck": ck, "sk": sk_,
            "ident": np.eye(128, dtype=BF16_NP),
        }
        in_maps.append(m)
    has_beta = bool(np.any(qb) or np.any(kb))
    if has_beta:
        # rope applied to beta: R(beta)[s, 2i] = b[2i] cos - b[2i+1] sin, etc.
        for c in range(N_CORES):
            e = c % E
            for name, beta in (("rbq", qb[e]), ("rbk", kb[e])):
                bs = np.tile(beta[None, :], (S, 1))
                rb = bs * cos_full + np.tile(
                    swap_pairs(beta)[None, :], (S, 1)
                ) * ssin_full
                in_maps[c][name] = rb.astype(np.float32)
    return in_maps, has_beta


def _trace(nc, tc, cfg, has_beta, dbg=False):
    from contextlib import ExitStack
    import concourse.bass as bass
    from concourse import mybir

    BF16 = mybir.dt.bfloat16
    F32 = mybir.dt.float32
    AF = mybir.ActivationFunctionType
    ALU = mybir.AluOpType

    S, D, H = cfg["S"], cfg["D"], cfg["H"]
    NB = D // 128            # d blocks
    NS = S // 128            # s tiles
    SQC = cfg["SQC"]         # sq chunk size for attention (<= 1024)
    NSQ = S // SQC
    NBN = (D + 511) // 512   # bn_stats chunks

    # ---- dram parameters
    xT = nc.dram_tensor("xT", [D, S], BF16, kind="ExternalInput")
    wqT = nc.dram_tensor("wqT", [D, D], BF16, kind="ExternalInput")
    wkT = nc.dram_tensor("wkT", [D, D], BF16, kind="ExternalInput")
    wvT = nc.dram_tensor("wvT", [D, D], BF16, kind="ExternalInput")
    woT = nc.dram_tensor("woT", [D, D], BF16, kind="ExternalInput")
    gw = nc.dram_tensor("gw", [D, 1], BF16, kind="ExternalInput")
    negb = nc.dram_tensor("negb", [1, 1], F32, kind="ExternalInput")
    cq_d = nc.dram_tensor("cq", [S, D], BF16, kind="ExternalInput")
    sq_d = nc.dram_tensor("sq", [S, D], BF16, kind="ExternalInput")
    ck_d = nc.dram_tensor("ck", [S, D], BF16, kind="ExternalInput")
    sk_d = nc.dram_tensor("sk", [S, D], BF16, kind="ExternalInput")
    id_d = nc.dram_tensor("ident", [128, 128], BF16, kind="ExternalInput")
    if has_beta:
        rbq_d = nc.dram_tensor("rbq", [S, D], F32, kind="ExternalInput")
        rbk_d = nc.dram_tensor("rbk", [S, D], F32, kind="ExternalInput")
    outT = nc.dram_tensor("outT", [D, S], F32, kind="ExternalOutput")
    if dbg:
        d_qT = nc.dram_tensor("d_qT", [128, NB, S], BF16, kind="ExternalOutput")
        d_kT = nc.dram_tensor("d_kT", [128, NB, S], BF16, kind="ExternalOutput")
        d_v = nc.dram_tensor("d_v", [128, S // 128, H, HD + 1], BF16, kind="ExternalOutput")
        d_gate = nc.dram_tensor("d_gate", [1, S], F32, kind="ExternalOutput")
        d_oT = nc.dram_tensor("d_oT", [128, NB, S], BF16, kind="ExternalOutput")
        d_gout = nc.dram_tensor("d_gout", [D, S], F32, kind="ExternalOutput")
        d_sc0 = nc.dram_tensor("d_sc0", [128, cfg["SQC"]], F32, kind="ExternalOutput")
        d_pt0 = nc.dram_tensor("d_pt0", [128, cfg["SQC"]], BF16, kind="ExternalOutput")
        d_ps0 = nc.dram_tensor("d_ps0", [HD + 1, cfg["SQC"]], F32, kind="ExternalOutput")

    groups = [[0, 1, 2, 3], [4, 5, 6, 7]]

    def mm(out, lhsT, rhs, start, stop, tile_position=None, step=512):
        """matmul with the moving/free dim split so PSUM writes stay in-bank."""
        n = out.shape[-1]
        for i0 in range(0, n, step):
            i1 = min(n, i0 + step)
            nc.tensor.matmul(
                out[:, i0:i1], lhsT, rhs[:, i0:i1],
                start=start, stop=stop, tile_position=tile_position,
            )

    ctx = ExitStack()
    with ctx:
        # ---- long-lived pools
        persist = ctx.enter_context(tc.tile_pool(name="persist", bufs=1))
        dram = ctx.enter_context(tc.tile_pool(name="dram", bufs=1, space="DRAM"))

        negb_sb = persist.tile([1, 1], F32, tag="negb")
        ident = persist.tile([128, 128], BF16, tag="ident")
        eps_t = persist.tile([128, 1], F32, tag="eps")
        qT_sb = persist.tile([128, NB, S], BF16, tag="qT")
        kT_sb = persist.tile([128, NB, S], BF16, tag="kT")
        v_all = persist.tile([128, NS, H, HD + 1], BF16, tag="v")
        gate_row = persist.tile([1, S], F32, tag="gate")
        ones_bc = persist.tile([HD + 1, 128], F32, tag="ones_bc")

        nc.sync.dma_start(negb_sb[:], negb[:])
        nc.sync.dma_start(ident[:], id_d[:])
        nc.vector.memset(eps_t[:], EPS)
        nc.vector.memset(v_all[:, :, :, HD:HD + 1], 1.0)
        nc.vector.memset(ones_bc[:], 1.0)

        # ================= Phase A: projections + LN + RoPE + transposes ====
        with (
            tc.tile_pool(name="wpool", bufs=1) as wpool,
            tc.tile_pool(name="xt", bufs=2) as xt_pool,
            tc.tile_pool(name="tabs", bufs=2) as tab_pool,
            tc.tile_pool(name="work", bufs=2) as work,
            tc.tile_pool(name="stats", bufs=2) as stats_pool,
            tc.tile_pool(name="ps_qkv", bufs=1, space="PSUM") as ps_qkv,
            tc.tile_pool(name="ps_g", bufs=1, space="PSUM") as ps_gate,
            tc.tile_pool(name="ps_t", bufs=1, space="PSUM") as ps_tp,
        ):
            wq_sb = wpool.tile([128, NB, D], BF16, tag="wq")
            wk_sb = wpool.tile([128, NB, D], BF16, tag="wk")
            wv_sb = wpool.tile([128, NB, D], BF16, tag="wv")
            gw_sb = wpool.tile([128, NB, 1], BF16, tag="gw")
            nc.sync.dma_start(wq_sb[:], wqT[:].rearrange("(j p) n -> p j n", p=128))
            nc.sync.dma_start(wk_sb[:], wkT[:].rearrange("(j p) n -> p j n", p=128))
            nc.sync.dma_start(wv_sb[:], wvT[:].rearrange("(j p) n -> p j n", p=128))
            nc.sync.dma_start(gw_sb[:], gw[:].rearrange("(j p) n -> p j n", p=128))
            for st in range(NS):
                s0 = st * 128
                xt = xt_pool.tile([128, NB, 128], BF16, tag="xt")
                nc.sync.dma_start(
                    xt[:], xT[:, s0:s0 + 128].rearrange("(j p) c -> p j c", p=128)
                )
                psq = ps_qkv.tile([128, D], F32, tag="psq")
                psk = ps_qkv.tile([128, D], F32, tag="psk")
                psv = ps_qkv.tile([128, D], F32, tag="psv")
                psg = ps_gate.tile([1, 128], F32, tag="psg")
                for j in range(NB):
                    fl = dict(start=(j == 0), stop=(j == NB - 1))
                    mm(psq[:], xt[:, j, :], wq_sb[:, j, :], **fl)
                    mm(psk[:], xt[:, j, :], wk_sb[:, j, :], **fl)
                    mm(psv[:], xt[:, j, :], wv_sb[:, j, :], **fl)
                    mm(psg[:], gw_sb[:, j, :], xt[:, j, :], **fl)

                # gate: sigmoid(z) = 1 / (1 + exp(-z - b))
                ge = stats_pool.tile([1, 128], F32, tag="ge")
                nc.scalar.activation(ge[:], psg[:], AF.Exp, scale=-1.0,
                                     bias=negb_sb[:])
                gp = stats_pool.tile([1, 128], F32, tag="gp")
                nc.vector.tensor_scalar_add(gp[:], ge[:], 1.0)
                nc.vector.reciprocal(gate_row[:, s0:s0 + 128], gp[:])

                # v staging: [128, H, HD] -> v_all[:, st, :, 0:HD]
                nc.vector.tensor_copy(
                    v_all[:, st, :, 0:HD],
                    psv[:].rearrange("p (h c) -> p h c", c=HD),
                )

                for name, ps, c_d, s_d in (
                    ("q", psq, cq_d, sq_d),
                    ("k", psk, ck_d, sk_d),
                ):
                    # LN stats (bn_stats is limited to 512 free elements per call)
                    stats = stats_pool.tile([128, NBN, 6], F32, tag="bnst")
                    for cbn in range(NBN):
                        f0 = cbn * 512
                        nc.vector.bn_stats(
                            stats[:, cbn, :], ps[:, f0:min(D, f0 + 512)]
                        )
                    aggr = stats_pool.tile([128, 2], F32, tag="bnag")
                    nc.vector.bn_aggr(aggr[:], stats[:])
                    lnv = stats_pool.tile([128, 1], F32, tag="lnv")
                    nc.scalar.activation(lnv[:], aggr[:, 1:2], AF.Ln, bias=eps_t[:])
                    istd = stats_pool.tile([128, 1], F32, tag="istd")
                    nc.scalar.activation(istd[:], lnv[:], AF.Exp, scale=-0.5)
                    negmu = stats_pool.tile([128, 1], F32, tag="negmu")
                    nc.vector.tensor_scalar_mul(negmu[:], aggr[:, 0:1], -1.0)
                    xn = work.tile([128, D], BF16, tag="xn")
                    nc.vector.tensor_scalar(
                        xn[:], ps[:], scalar1=negmu[:], scalar2=istd[:],
                        op0=ALU.add, op1=ALU.mult,
                    )
                    # rope
                    ct = tab_pool.tile([128, D], BF16, tag="ct")
                    nc.sync.dma_start(ct[:], c_d[s0:s0 + 128, :])
                    sst = tab_pool.tile([128, D], BF16, tag="sst")
                    nc.sync.dma_start(sst[:], s_d[s0:s0 + 128, :])
                    t1 = work.tile([128, D], BF16, tag="t1")
                    nc.vector.tensor_tensor(t1[:], xn[:], ct[:], op=ALU.mult)
                    t2 = work.tile([128, D], BF16, tag="t2")
                    xn_sw = xn[:].rearrange("p (c two) -> p c two", two=2)[:, :, ::-1]
                    nc.vector.tensor_tensor(
                        t2[:].rearrange("p (c two) -> p c two", two=2),
                        xn_sw,
                        sst[:].rearrange("p (c two) -> p c two", two=2),
                        op=ALU.mult,
                    )
                    xr = work.tile([128, D], BF16, tag="xr")
                    if has_beta:
                        rb_t = tab_pool.tile([128, D], F32, tag="rb")
                        nc.sync.dma_start(
                            rb_t[:], (rbq_d if name == "q" else rbk_d)[s0:s0 + 128, :]
                        )
                        t3 = work.tile([128, D], BF16, tag="t3")
                        nc.vector.tensor_tensor(t3[:], t1[:], t2[:], op=ALU.add)
                        nc.vector.tensor_tensor(xr[:], t3[:], rb_t[:], op=ALU.add)
                    else:
                        nc.vector.tensor_tensor(xr[:], t1[:], t2[:], op=ALU.add)
                    # transpose to [d, s]
                    dst = qT_sb if name == "q" else kT_sb
                    TG = 4 if NB % 4 == 0 else NB
                    for g0 in range(0, NB, TG):
                        tp = ps_tp.tile([128, TG * 128], BF16, tag="tp")
                        for j2 in range(TG):
                            nc.tensor.transpose(
                                tp[:, j2 * 128:(j2 + 1) * 128],
                                xr[:, (g0 + j2) * 128:(g0 + j2 + 1) * 128],
                                ident[:],
                            )
                        nc.vector.tensor_copy(
                            dst[:, g0:g0 + TG, s0:s0 + 128],
                            tp[:].rearrange("p (j c) -> p j c", c=128),
                        )

        # ================= Phase B: attention per head =======================
        late = ctx.enter_context(tc.tile_pool(name="late", bufs=1))
        oT_sb = late.tile([128, NB, S], BF16, tag="oT")
        with (
            tc.tile_pool(name="pt", bufs=3) as pt_pool,
            tc.tile_pool(name="nrm", bufs=2) as nrm_pool,
            tc.tile_pool(name="ps_s", bufs=2, space="PSUM") as ps_sc,
            tc.tile_pool(name="ps_o", bufs=1, space="PSUM") as ps_ot,
            tc.tile_pool(name="ps_b", bufs=1, space="PSUM") as ps_bc,
        ):
            for h in range(H):
                jb, off = h // 2, (h % 2) * 64
                tp_arg = (off, 0) if off else None
                for sqh in range(NSQ):
                    sq0 = sqh * SQC
                    ps_o = ps_ot.tile([HD + 1, SQC], F32, tag="pso")
                    for skt in range(NS):
                        ps_s = ps_sc.tile([128, SQC], F32, tag="pss")
                        mm(
                            ps_s[:],
                            kT_sb[off:off + 64, jb, skt * 128:(skt + 1) * 128],
                            qT_sb[off:off + 64, jb, sq0:sq0 + SQC],
                            start=True, stop=True, tile_position=tp_arg,
                        )
                        pt = pt_pool.tile([128, SQC], BF16, tag="pt")
                        nc.scalar.activation(pt[:], ps_s[:], AF.Exp, scale=0.125)
                        if dbg and h == 0 and sqh == 0 and skt == 0:
                            scp = pt_pool.tile([128, SQC], F32, tag="scp")
                            nc.vector.tensor_copy(scp[:], ps_s[:])
                            nc.sync.dma_start(d_sc0[:], scp[:])
                            nc.sync.dma_start(d_pt0[:], pt[:])
                        mm(
                            ps_o[:], v_all[:, skt, h, :], pt[:],
                            start=(skt == 0), stop=(skt == NS - 1),
                        )
                    if dbg and h == 0 and sqh == 0:
                        psp = nrm_pool.tile([HD + 1, SQC], F32, tag="psp")
                        nc.vector.tensor_copy(psp[:], ps_o[:])
                        nc.sync.dma_start(d_ps0[:], psp[:])
                    inv = nrm_pool.tile([HD + 1, SQC], F32, tag="inv")
                    nc.vector.reciprocal(inv[HD:HD + 1, :], ps_o[HD:HD + 1, :])
                    bps = ps_bc.tile([64, SQC], F32, tag="bps")
                    mm(bps[:], ones_bc[HD:HD + 1, 0:64], inv[HD:HD + 1, :],
                       start=True, stop=True)
                    binv = nrm_pool.tile([64, SQC], F32, tag="binv")
                    nc.vector.tensor_copy(binv[:], bps[:])
                    if off == 0:
                        nc.vector.tensor_tensor(
                            oT_sb[0:64, jb, sq0:sq0 + SQC],
                            ps_o[0:64, :], binv[:], op=ALU.mult,
                        )
                    else:
                        stag = nrm_pool.tile([64, SQC], BF16, tag="stag")
                        nc.vector.tensor_tensor(
                            stag[:], ps_o[0:64, :], binv[:], op=ALU.mult
                        )
                        nc.sync.dma_start(oT_sb[64:128, jb, sq0:sq0 + SQC], stag[:])

        if dbg:
            nc.sync.dma_start(d_qT[:], qT_sb[:])
            nc.sync.dma_start(d_kT[:], kT_sb[:])
            nc.sync.dma_start(d_v[:], v_all[:])
            nc.sync.dma_start(d_gate[:], gate_row[:])
            nc.sync.dma_start(d_oT[:], oT_sb[:])
        # ================= Phase C: output projection + gate + allreduce =====
        with (
            tc.tile_pool(name="wo", bufs=1) as wo_pool,
            tc.tile_pool(name="go", bufs=2) as go_pool,
            tc.tile_pool(name="ps_f", bufs=2, space="PSUM") as ps_fin,
            tc.tile_pool(name="ps_bg", bufs=1, space="PSUM") as ps_bgp,
        ):
            wo_sb = wo_pool.tile([128, NB, D], BF16, tag="wo")
            nc.sync.dma_start(wo_sb[:], woT[:].rearrange("(j p) n -> p j n", p=128))
            bg = wo_pool.tile([128, S], F32, tag="bg")
            for sqh in range(NSQ):
                sq0 = sqh * SQC
                bgp = ps_bgp.tile([128, SQC], F32, tag="bgp")
                mm(bgp[:], ones_bc[0:1, 0:128], gate_row[:, sq0:sq0 + SQC],
                   start=True, stop=True)
                nc.vector.tensor_copy(bg[:, sq0:sq0 + SQC], bgp[:])
            gout = dram.tile([D, S], F32, tag="gout")
            red = dram.tile([D, S], F32, tag="red")
            for db in range(NB):
                for sqh in range(NSQ):
                    sq0 = sqh * SQC
                    psf = ps_fin.tile([128, SQC], F32, tag="psf")
                    for j in range(NB):
                        mm(
                            psf[:],
                            wo_sb[:, j, db * 128:(db + 1) * 128],
                            oT_sb[:, j, sq0:sq0 + SQC],
                            start=(j == 0), stop=(j == NB - 1),
                        )
                    gs = go_pool.tile([128, SQC], F32, tag="gs")
                    nc.vector.tensor_tensor(
                        gs[:], psf[:], bg[:, sq0:sq0 + SQC], op=ALU.mult
                    )
                    nc.sync.dma_start(
                        gout[db * 128:(db + 1) * 128, sq0:sq0 + SQC], gs[:]
                    )
            if dbg:
                nc.sync.dma_start(d_gout[:], gout[:])
            nc.gpsimd.collective_compute(
                "AllReduce",
                mybir.AluOpType.add,
                replica_groups=groups,
                ins=[gout.opt()],
                outs=[red.opt()],
            )
            nc.sync.dma_start(outT[:], red[:])


def _run(inputs, cfg=None, trace=False, trace_kwargs=None, dbg=False):
    import concourse.tile as tile
    from concourse import bacc
    import concourse.bass_utils as bass_utils

    if cfg is None:
        cfg = {"B": B, "S": S, "D": D, "E": E, "H": H, "SQC": 1024}

    in_maps, has_beta = _host_prep(inputs, cfg)

    nc = bacc.Bacc("TRN2", target_bir_lowering=False, debug=False,
                   num_devices=N_CORES)
    with tile.TileContext(nc) as tc:
        _trace(nc, tc, cfg, has_beta, dbg=dbg)
    nc.compile()

    res = bass_utils.run_bass_kernel_spmd(
        nc, in_maps, list(range(N_CORES)), trace=trace,
        **(trace_kwargs or {}),
    )
    Bc, Sc, Dc = cfg["B"], cfg["S"], cfg["D"]
    out = np.empty((Bc, Sc, Dc), np.float32)
    for b in range(Bc):
        out[b] = res.results[b * 4]["outT"].T
    return out, res


def kernel(**inputs):
    out, _ = _run(inputs)
    return out


# revision 15
# speedup vs baseline: 1.5450x; 1.5450x over previous
"""MoE attention kernel for Trainium2 (8 NeuronCores via bass/Tile).

Sharding: core c -> (expert e = c % 4, batch b = c // 4). Each core computes
its expert's full attention for its batch, applies the sigmoid gate, and the
gated partial outputs are AllReduced within each batch group {0..3}, {4..7}.

All matmuls run in bf16 (fp32 PSUM accumulation). Layouts:
  - x is fed transposed per batch: xT [D, S]
  - weights are fed transposed: w*T [D_in, D_out]
  - q/k are computed in natural [s, d] layout (for layernorm + rope), then
    transposed on the PE to qT/kT [d, s] for the attention matmuls
  - attention computes P^T = exp(scoresT) [sk, sq]; an extra all-ones column
    in the stationary [v | 1] operand accumulates the softmax denominator
  - output projection produces outT [D, S]; host transposes back
"""
import sys
import numpy as np

sys.path.insert(0, "/opt/trn_rl_repo")

import ml_dtypes  # noqa: E402

BF16_NP = ml_dtypes.bfloat16

# problem config (full size, hardcoded for the grader)
B, S, D, E, H = 2, 2048, 1024, 4, 16
HD = 64
N_CORES = 8
EPS = 1e-5


def _host_prep(inputs, cfg):
    """Build per-core input maps (numpy only)."""
    B, S, D, E, H = cfg["B"], cfg["S"], cfg["D"], cfg["E"], cfg["H"]
    x = np.asarray(inputs["x"], np.float32)
    fc = np.asarray(inputs["freqs_cos"], np.float32)  # [S, HD//2]
    fs = np.asarray(inputs["freqs_sin"], np.float32)
    wq, wk, wv, wo = (np.asarray(inputs[n], np.float32) for n in ("wq", "wk", "wv", "wo"))
    qg, qb = np.asarray(inputs["q_gamma"], np.float32), np.asarray(inputs["q_beta"], np.float32)
    kg, kb = np.asarray(inputs["k_gamma"], np.float32), np.asarray(inputs["k_beta"], np.float32)
    gw, gb = np.asarray(inputs["gate_w"], np.float32), np.asarray(inputs["gate_b"], np.float32)

    # expanded rope tables [S, D]: cos/sin duplicated into feature pairs, tiled over heads
    nh2 = D // (2 * fc.shape[1])  # number of head-blocks the [S, hd] pattern tiles over
    cos2 = np.repeat(fc, 2, axis=1)  # [S, hd]
    sin2 = np.repeat(fs, 2, axis=1)
    sgn = np.tile(np.array([-1.0, 1.0], np.float32), fc.shape[1])  # [-s,+s] pairs
    cos_full = np.tile(cos2, (1, nh2))  # [S, D]
    ssin_full = np.tile(sin2 * sgn[None, :], (1, nh2))  # signed sin [S, D]

    def swap_pairs(v):
        return v.reshape(-1, 2)[:, ::-1].reshape(-1)

    in_maps = []
    for c in range(N_CORES):
        e, b = c % E, c // E
        # fold gamma into rope tables: C' = cos * gamma ; S' = ssin * gamma[swap]
        cq = (cos_full * qg[e][None, :]).astype(BF16_NP)
        sq = (ssin_full * swap_pairs(qg[e])[None, :]).astype(BF16_NP)
        ck = (cos_full * kg[e][None, :]).astype(BF16_NP)
        sk_ = (ssin_full * swap_pairs(kg[e])[None, :]).astype(BF16_NP)
        m = {
            "xT": np.ascontiguousarray(x[b].T).astype(BF16_NP),
            "wqT": np.ascontiguousarray(wq[e].T).astype(BF16_NP),
            "wkT": np.ascontiguousarray(wk[e].T).astype(BF16_NP),
            "wvT": np.ascontiguousarray(wv[e].T).astype(BF16_NP),
            "woT": np.ascontiguousarray(wo[e].T).astype(BF16_NP),
            "gw": np.ascontiguousarray(gw[e][:, None]).astype(BF16_NP),
            "negb": np.full((1, 1), -gb[e], np.float32),
            "cq": cq, "sq": sq, "ck": ck, "sk": sk_,
            "ident": np.eye(128, dtype=BF16_NP),
            "sel2": np.repeat(np.eye(2, dtype=BF16_NP), 64, axis=1),
        }
        in_maps.append(m)
    has_beta = bool(np.any(qb) or np.any(kb))
    if has_beta:
        # rope applied to beta: R(beta)[s, 2i] = b[2i] cos - b[2i+1] sin, etc.
        for c in range(N_CORES):
            e = c % E
            for name, beta in (("rbq", qb[e]), ("rbk", kb[e])):
                bs = np.tile(beta[None, :], (S, 1))
                rb = bs * cos_full + np.tile(
                    swap_pairs(beta)[None, :], (S, 1)
                ) * ssin_full
                in_maps[c][name] = rb.astype(np.float32)
    return in_maps, has_beta


def _trace(nc, tc, cfg, has_beta, dbg=False):
    from contextlib import ExitStack
    import concourse.bass as bass
    from concourse import mybir

    BF16 = mybir.dt.bfloat16
    F32 = mybir.dt.float32
    AF = mybir.ActivationFunctionType
    ALU = mybir.AluOpType

    S, D, H = cfg["S"], cfg["D"], cfg["H"]
    NB = D // 128            # d blocks
    NS = S // 128            # s tiles
    SQC = cfg["SQC"]         # sq chunk size for attention (<= 1024)
    NSQ = S // SQC
    NBN = (D + 511) // 512   # bn_stats chunks

    # ---- dram parameters
    xT = nc.dram_tensor("xT", [D, S], BF16, kind="ExternalInput")
    wqT = nc.dram_tensor("wqT", [D, D], BF16, kind="ExternalInput")
    wkT = nc.dram_tensor("wkT", [D, D], BF16, kind="ExternalInput")
    wvT = nc.dram_tensor("wvT", [D, D], BF16, kind="ExternalInput")
    woT = nc.dram_tensor("woT", [D, D], BF16, kind="ExternalInput")
    gw = nc.dram_tensor("gw", [D, 1], BF16, kind="ExternalInput")
    negb = nc.dram_tensor("negb", [1, 1], F32, kind="ExternalInput")
    cq_d = nc.dram_tensor("cq", [S, D], BF16, kind="ExternalInput")
    sq_d = nc.dram_tensor("sq", [S, D], BF16, kind="ExternalInput")
    ck_d = nc.dram_tensor("ck", [S, D], BF16, kind="ExternalInput")
    sk_d = nc.dram_tensor("sk", [S, D], BF16, kind="ExternalInput")
    id_d = nc.dram_tensor("ident", [128, 128], BF16, kind="ExternalInput")
    sel2_d = nc.dram_tensor("sel2", [2, 128], BF16, kind="ExternalInput")
    if has_beta:
        rbq_d = nc.dram_tensor("rbq", [S, D], F32, kind="ExternalInput")
        rbk_d = nc.dram_tensor("rbk", [S, D], F32, kind="ExternalInput")
    DS = D // 4  # ReduceScatter shard rows per core
    outT = nc.dram_tensor("outT", [DS, S], F32, kind="ExternalOutput")
    if dbg:
        d_qT = nc.dram_tensor("d_qT", [128, NB, S], BF16, kind="ExternalOutput")
        d_kT = nc.dram_tensor("d_kT", [128, NB, S], BF16, kind="ExternalOutput")
        d_v = nc.dram_tensor("d_v", [128, S // 128, H, HD + 1], BF16, kind="ExternalOutput")
        d_gate = nc.dram_tensor("d_gate", [1, S], F32, kind="ExternalOutput")
        d_oT = nc.dram_tensor("d_oT", [128, NB, S], BF16, kind="ExternalOutput")
        d_gout = nc.dram_tensor("d_gout", [D, S], F32, kind="ExternalOutput")
        d_sc0 = nc.dram_tensor("d_sc0", [128, cfg["SQC"]], F32, kind="ExternalOutput")
        d_pt0 = nc.dram_tensor("d_pt0", [128, cfg["SQC"]], BF16, kind="ExternalOutput")
        d_ps0 = nc.dram_tensor("d_ps0", [HD + 1, cfg["SQC"]], F32, kind="ExternalOutput")

    groups = [[0, 1, 2, 3], [4, 5, 6, 7]]

    def mm(out, lhsT, rhs, start, stop, tile_position=None, step=512):
        """matmul with the moving/free dim split so PSUM writes stay in-bank."""
        n = out.shape[-1]
        for i0 in range(0, n, step):
            i1 = min(n, i0 + step)
            nc.tensor.matmul(
                out[:, i0:i1], lhsT, rhs[:, i0:i1],
                start=start, stop=stop, tile_position=tile_position,
            )

    ctx = ExitStack()
    with ctx:
        # ---- long-lived pools
        persist = ctx.enter_context(tc.tile_pool(name="persist", bufs=1))
        dram = ctx.enter_context(tc.tile_pool(name="dram", bufs=1, space="DRAM"))

        negb_sb = persist.tile([1, 1], F32, tag="negb")
        ident = persist.tile([128, 128], BF16, tag="ident")
        eps_t = persist.tile([128, 1], F32, tag="eps")
        qT_sb = persist.tile([128, NB, S], BF16, tag="qT")
        kT_sb = persist.tile([128, NB, S], BF16, tag="kT")
        v_all = persist.tile([128, NS, H, HD + 1], BF16, tag="v")
        gate_row = persist.tile([1, S], F32, tag="gate")
        ones_bc = persist.tile([1, 128], BF16, tag="ones_bc")
        sel2_sb = persist.tile([2, 128], BF16, tag="sel2")

        nc.sync.dma_start(negb_sb[:], negb[:])
        nc.sync.dma_start(ident[:], id_d[:])
        nc.vector.memset(eps_t[:], EPS)
        nc.vector.memset(v_all[:, :, :, HD:HD + 1], 1.0)
        nc.vector.memset(ones_bc[:], 1.0)
        nc.sync.dma_start(sel2_sb[:], sel2_d[:])

        # ================= Phase A: projections + LN + RoPE + transposes ====
        with (
            tc.tile_pool(name="wpool", bufs=1) as wpool,
            tc.tile_pool(name="xt", bufs=2) as xt_pool,
            tc.tile_pool(name="tabs", bufs=2) as tab_pool,
            tc.tile_pool(name="work", bufs=2) as work,
            tc.tile_pool(name="stats", bufs=2) as stats_pool,
            tc.tile_pool(name="ps_qkv", bufs=1, space="PSUM") as ps_qkv,
            tc.tile_pool(name="ps_g", bufs=1, space="PSUM") as ps_gate,
            tc.tile_pool(name="ps_t", bufs=1, space="PSUM") as ps_tp,
        ):
            wq_sb = wpool.tile([128, NB, D], BF16, tag="wq")
            wk_sb = wpool.tile([128, NB, D], BF16, tag="wk")
            wv_sb = wpool.tile([128, NB, D], BF16, tag="wv")
            gw_sb = wpool.tile([128, NB, 1], BF16, tag="gw")
            nc.sync.dma_start(wq_sb[:], wqT[:].rearrange("(j p) n -> p j n", p=128))
            nc.sync.dma_start(wk_sb[:], wkT[:].rearrange("(j p) n -> p j n", p=128))
            nc.sync.dma_start(wv_sb[:], wvT[:].rearrange("(j p) n -> p j n", p=128))
            nc.sync.dma_start(gw_sb[:], gw[:].rearrange("(j p) n -> p j n", p=128))
            for st in range(NS):
                s0 = st * 128
                xt = xt_pool.tile([128, NB, 128], BF16, tag="xt")
                nc.sync.dma_start(
                    xt[:], xT[:, s0:s0 + 128].rearrange("(j p) c -> p j c", p=128)
                )
                psq = ps_qkv.tile([128, D], F32, tag="psq")
                psk = ps_qkv.tile([128, D], F32, tag="psk")
                psv = ps_qkv.tile([128, D], F32, tag="psv")
                psg = ps_gate.tile([1, 128], F32, tag="psg")
                for j in range(NB):
                    fl = dict(start=(j == 0), stop=(j == NB - 1))
                    mm(psq[:], xt[:, j, :], wq_sb[:, j, :], **fl)
                    mm(psk[:], xt[:, j, :], wk_sb[:, j, :], **fl)
                    mm(psv[:], xt[:, j, :], wv_sb[:, j, :], **fl)
                    mm(psg[:], gw_sb[:, j, :], xt[:, j, :], **fl)

                # v staging: [128, H, HD] -> v_all[:, st, :, 0:HD]
                nc.vector.tensor_copy(
                    v_all[:, st, :, 0:HD],
                    psv[:].rearrange("p (h c) -> p h c", c=HD),
                )

                # LN stats for q and k together -> one Ln + one Exp (batched
                # so the ACT table set switches only twice per s-tile)
                aggr = stats_pool.tile([128, 2, 2], F32, tag="bnag")
                for ti, ps in ((0, psq), (1, psk)):
                    stats = stats_pool.tile([128, NBN, 6], F32, tag=f"bnst{ti}")
                    for cbn in range(NBN):
                        f0 = cbn * 512
                        nc.vector.bn_stats(
                            stats[:, cbn, :], ps[:, f0:min(D, f0 + 512)]
                        )
                    nc.vector.bn_aggr(aggr[:, ti, :], stats[:])
                lnv = stats_pool.tile([128, 2], F32, tag="lnv")
                nc.scalar.activation(lnv[:], aggr[:, :, 1], AF.Ln, bias=eps_t[:])
                istd = stats_pool.tile([128, 2], F32, tag="istd")
                nc.scalar.activation(istd[:], lnv[:], AF.Exp, scale=-0.5)
                # gate: sigmoid(z) = 1 / (1 + exp(-z - b)) (Exp adjacent to above)
                ge = stats_pool.tile([1, 128], F32, tag="ge")
                nc.scalar.activation(ge[:], psg[:], AF.Exp, scale=-1.0,
                                     bias=negb_sb[:])
                gp = stats_pool.tile([1, 128], F32, tag="gp")
                nc.vector.tensor_scalar_add(gp[:], ge[:], 1.0)
                nc.vector.reciprocal(gate_row[:, s0:s0 + 128], gp[:])
                negmu = stats_pool.tile([128, 2], F32, tag="negmu")
                nc.vector.tensor_scalar_mul(negmu[:], aggr[:, :, 0], -1.0)

                for ti, name, ps, c_d, s_d in (
                    (0, "q", psq, cq_d, sq_d),
                    (1, "k", psk, ck_d, sk_d),
                ):
                    xn = work.tile([128, D], BF16, tag="xn")
                    nc.vector.tensor_scalar(
                        xn[:], ps[:], scalar1=negmu[:, ti:ti + 1],
                        scalar2=istd[:, ti:ti + 1],
                        op0=ALU.add, op1=ALU.mult,
                    )
                    # rope
                    ct = tab_pool.tile([128, D], BF16, tag="ct")
                    nc.sync.dma_start(ct[:], c_d[s0:s0 + 128, :])
                    sst = tab_pool.tile([128, D], BF16, tag="sst")
                    nc.sync.dma_start(sst[:], s_d[s0:s0 + 128, :])
                    t1 = work.tile([128, D], BF16, tag="t1")
                    nc.vector.tensor_tensor(t1[:], xn[:], ct[:], op=ALU.mult)
                    t2 = work.tile([128, D], BF16, tag="t2")
                    xn_sw = xn[:].rearrange("p (c two) -> p c two", two=2)[:, :, ::-1]
                    nc.vector.tensor_tensor(
                        t2[:].rearrange("p (c two) -> p c two", two=2),
                        xn_sw,
                        sst[:].rearrange("p (c two) -> p c two", two=2),
                        op=ALU.mult,
                    )
                    xr = work.tile([128, D], BF16, tag="xr")
                    if has_beta:
                        rb_t = tab_pool.tile([128, D], F32, tag="rb")
                        nc.sync.dma_start(
                            rb_t[:], (rbq_d if name == "q" else rbk_d)[s0:s0 + 128, :]
                        )
                        t3 = work.tile([128, D], BF16, tag="t3")
                        nc.vector.tensor_tensor(t3[:], t1[:], t2[:], op=ALU.add)
                        nc.vector.tensor_tensor(xr[:], t3[:], rb_t[:], op=ALU.add)
                    else:
                        nc.vector.tensor_tensor(xr[:], t1[:], t2[:], op=ALU.add)
                    # transpose to [d, s]
                    dst = qT_sb if name == "q" else kT_sb
                    TG = 4 if NB % 4 == 0 else NB
                    for g0 in range(0, NB, TG):
                        tp = ps_tp.tile([128, TG * 128], BF16, tag="tp")
                        for j2 in range(TG):
                            nc.tensor.transpose(
                                tp[:, j2 * 128:(j2 + 1) * 128],
                                xr[:, (g0 + j2) * 128:(g0 + j2 + 1) * 128],
                                ident[:],
                            )
                        nc.vector.tensor_copy(
                            dst[:, g0:g0 + TG, s0:s0 + 128],
                            tp[:].rearrange("p (j c) -> p j c", c=128),
                        )

        # ================= Phase B: attention per head =======================
        late = ctx.enter_context(tc.tile_pool(name="late", bufs=1))
        oT_sb = late.tile([128, NB, S], BF16, tag="oT")
        den_all = late.tile([H, S], F32, tag="den")
        with (
            tc.tile_pool(name="pt", bufs=3) as pt_pool,
            tc.tile_pool(name="nrm", bufs=2) as nrm_pool,
            tc.tile_pool(name="ps_s", bufs=2, space="PSUM") as ps_sc,
            tc.tile_pool(name="ps_o", bufs=2, space="PSUM") as ps_ot,
        ):
            for h in range(H):
                jb, off = h // 2, (h % 2) * 64
                tp_arg = (off, 0) if off else None
                for sqh in range(NSQ):
                    sq0 = sqh * SQC
                    ps_o = ps_ot.tile([HD + 1, SQC], F32, tag="pso")
                    for skt in range(NS):
                        ps_s = ps_sc.tile([128, SQC], F32, tag="pss")
                        mm(
                            ps_s[:],
                            kT_sb[off:off + 64, jb, skt * 128:(skt + 1) * 128],
                            qT_sb[off:off + 64, jb, sq0:sq0 + SQC],
                            start=True, stop=True, tile_position=tp_arg,
                        )
                        pt = pt_pool.tile([128, SQC], BF16, tag="pt")
                        nc.scalar.activation(pt[:], ps_s[:], AF.Exp, scale=0.125)
                        if dbg and h == 0 and sqh == 0 and skt == 0:
                            scp = pt_pool.tile([128, SQC], F32, tag="scp")
                            nc.vector.tensor_copy(scp[:], ps_s[:])
                            nc.sync.dma_start(d_sc0[:], scp[:])
                            nc.sync.dma_start(d_pt0[:], pt[:])
                        mm(
                            ps_o[:], v_all[:, skt, h, :], pt[:],
                            start=(skt == 0), stop=(skt == NS - 1),
                        )
                    if dbg and h == 0 and sqh == 0:
                        psp = nrm_pool.tile([HD + 1, SQC], F32, tag="psp")
                        nc.vector.tensor_copy(psp[:], ps_o[:])
                        nc.sync.dma_start(d_ps0[:], psp[:])
                    den_row = nrm_pool.tile([HD + 1, SQC], F32, tag="denrow")
                    nc.vector.tensor_copy(den_row[HD:HD + 1, :], ps_o[HD:HD + 1, :])
                    nc.sync.dma_start(den_all[h:h + 1, sq0:sq0 + SQC],
                                      den_row[HD:HD + 1, :])
                    if off == 0:
                        nc.vector.tensor_copy(
                            oT_sb[0:64, jb, sq0:sq0 + SQC], ps_o[0:64, :]
                        )
                    else:
                        stag = nrm_pool.tile([64, SQC], BF16, tag="stag")
                        nc.vector.tensor_copy(stag[:], ps_o[0:64, :])
                        nc.sync.dma_start(oT_sb[64:128, jb, sq0:sq0 + SQC], stag[:])

        # normalize all heads at once: inv = 1/den (one wide reciprocal),
        # broadcast per d-block with a K=2 selector matmul, multiply in place
        with (
            tc.tile_pool(name="nrm2", bufs=2) as nrm2,
            tc.tile_pool(name="ps_n", bufs=2, space="PSUM") as ps_n,
        ):
            inv_all = nrm2.tile([H, S], F32, tag="invall")
            nc.vector.reciprocal(inv_all[:], den_all[:])
            inv_bf = nrm2.tile([H, S], BF16, tag="invbf")
            nc.vector.tensor_copy(inv_bf[:], inv_all[:])
            for jb in range(NB):
                iv = nrm2.tile([2, S], BF16, tag="iv")
                nc.sync.dma_start(iv[0:1, :], inv_bf[2 * jb:2 * jb + 1, :])
                nc.sync.dma_start(iv[1:2, :], inv_bf[2 * jb + 1:2 * jb + 2, :])
                for sqh in range(NSQ):
                    sq0 = sqh * SQC
                    bf = ps_n.tile([128, SQC], F32, tag="bf")
                    mm(bf[:], sel2_sb[:], iv[0:2, sq0:sq0 + SQC],
                       start=True, stop=True)
                    nc.vector.tensor_tensor(
                        oT_sb[:, jb, sq0:sq0 + SQC],
                        oT_sb[:, jb, sq0:sq0 + SQC], bf[:], op=ALU.mult,
                    )

        if dbg:
            nc.sync.dma_start(d_qT[:], qT_sb[:])
            nc.sync.dma_start(d_kT[:], kT_sb[:])
            nc.sync.dma_start(d_v[:], v_all[:])
            nc.sync.dma_start(d_gate[:], gate_row[:])
            nc.sync.dma_start(d_oT[:], oT_sb[:])
        # ================= Phase C: output projection + gate + allreduce =====
        with (
            tc.tile_pool(name="wo", bufs=1) as wo_pool,
            tc.tile_pool(name="go", bufs=2) as go_pool,
            tc.tile_pool(name="ps_f", bufs=2, space="PSUM") as ps_fin,
            tc.tile_pool(name="ps_bg", bufs=1, space="PSUM") as ps_bgp,
        ):
            wo_sb = wo_pool.tile([128, NB, D], BF16, tag="wo")
            nc.sync.dma_start(wo_sb[:], woT[:].rearrange("(j p) n -> p j n", p=128))
            bg = wo_pool.tile([128, S], F32, tag="bg")
            gate_bf = wo_pool.tile([1, S], BF16, tag="gate_bf")
            nc.vector.tensor_copy(gate_bf[:], gate_row[:])
            for sqh in range(NSQ):
                sq0 = sqh * SQC
                bgp = ps_bgp.tile([128, SQC], F32, tag="bgp")
                mm(bgp[:], ones_bc[0:1, 0:128], gate_bf[:, sq0:sq0 + SQC],
                   start=True, stop=True)
                nc.vector.tensor_copy(bg[:, sq0:sq0 + SQC], bgp[:])
            gout = dram.tile([D, S], F32, tag="gout")
            red = dram.tile([DS, S], F32, tag="red")
            for db in range(NB):
                for sqh in range(NSQ):
                    sq0 = sqh * SQC
                    psf = ps_fin.tile([128, SQC], F32, tag="psf")
                    for j in range(NB):
                        mm(
                            psf[:],
                            wo_sb[:, j, db * 128:(db + 1) * 128],
                            oT_sb[:, j, sq0:sq0 + SQC],
                            start=(j == 0), stop=(j == NB - 1),
                        )
                    gs = go_pool.tile([128, SQC], F32, tag="gs")
                    nc.vector.tensor_tensor(
                        gs[:], psf[:], bg[:, sq0:sq0 + SQC], op=ALU.mult
                    )
                    nc.sync.dma_start(
                        gout[db * 128:(db + 1) * 128, sq0:sq0 + SQC], gs[:]
                    )
            if dbg:
                nc.sync.dma_start(d_gout[:], gout[:])
            nc.gpsimd.collective_compute(
                "ReduceScatter",
                mybir.AluOpType.add,
                replica_groups=groups,
                ins=[gout.opt()],
                outs=[red.opt()],
            )
            nc.sync.dma_start(outT[:], red[:])


def _run(inputs, cfg=None, trace=False, trace_kwargs=None, dbg=False):
    import concourse.tile as tile
    from concourse import bacc
    import concourse.bass_utils as bass_utils

    if cfg is None:
        cfg = {"B": B, "S": S, "D": D, "E": E, "H": H, "SQC": 1024}

    in_maps, has_beta = _host_prep(inputs, cfg)

    nc = bacc.Bacc("TRN2", target_bir_lowering=False, debug=False,
                   num_devices=N_CORES)
    with tile.TileContext(nc) as tc:
        _trace(nc, tc, cfg, has_beta, dbg=dbg)
    nc.compile()

    res = bass_utils.run_bass_kernel_spmd(
        nc, in_maps, list(range(N_CORES)), trace=trace,
        **(trace_kwargs or {}),
    )
    Bc, Sc, Dc = cfg["B"], cfg["S"], cfg["D"]
    out = np.empty((Bc, Sc, Dc), np.float32)
    for b in range(Bc):
        shard = np.concatenate(
            [res.results[b * 4 + i]["outT"] for i in range(4)], axis=0
        )
        out[b] = shard.T
    return out, res


def kernel(**inputs):
    out, _ = _run(inputs)
    return out
